# revision 2
# baseline (speedup 1.0000x reference)
# Trainium2 Bass kernel for the KerasLMU problem.
#
# Math: per time step t (T=1024 steps),
#   u_t = x_t @ e_x                       (B,1)
#   m_t = m_{t-1} @ A.T + b_row * u_t     (B,256)   -- linear recurrence
#   h_t = lrelu(x_t @ W_x + h_{t-1} @ W_h.T + m_t @ W_m.T)
#
# Reformulation:
#   m_t = sum_k A^k b u_{t-k}  (causal convolution), so
#   c_t := x_t @ W_x + m_t @ W_m.T = x_t @ W_x + sum_k G[k] u_{t-k}
#   with G[k] = W_m @ (A^k b) precomputed host-side in float64.
# The only sequential device work left is h_t = lrelu(c_t + h_{t-1} @ W_h.T).
#
# v2 changes vs baseline:
#   - whole conv pipeline in bf16 (fp32 matmul is 4 cyc/row on PE; bf16 is 1)
#   - c kept resident in SBUF (bf16, 64KB/partition): no cT DRAM roundtrip,
#     no per-block gather DMAs in the sequential phase
#   - x transposed host-side (no on-device transpose phase)
#   - sequential phase: 16 MMs/step issued in a latency-aware slot order
#     (each h-chunk finished early / read late), leaky-relu done per-chunk
#     as one DVE scalar_tensor_tensor (max(ps, 0.2*ps)), c prefetch for the
#     next step done on the otherwise-idle ScalarE.
#
# Sharding: data-parallel over batch. 64 batch rows -> 8 cores x 8 rows.
# All weights replicated; no collectives.

import os
import sys

sys.path.insert(0, "/opt/trn_rl_repo")

import numpy as np
import ml_dtypes

import concourse.bass as bass
import concourse.tile as tile
from concourse import bacc, mybir
from concourse.bass_utils import run_bass_kernel_spmd

F32 = mybir.dt.float32
BF16 = mybir.dt.bfloat16
BF = ml_dtypes.bfloat16

NCORES = 8
BATCH = 64
BC = BATCH // NCORES          # batch rows per core = 8
FEAT = 128
HID = 512
ORDER = 256
TFULL = 1024
TBLK = 64                     # seq-loop steps per output-DMA block

# Per-step matmul issue order: (read h-chunk r, write psum chunk w).
# Chunk completion slots: w0@7, w1@11, w2@13, w3@15; first reads:
# r0@0, r1@3, r2@5, r3@7 -- finished-early / read-late maximizes overlap
# of the PSUM->activation->SBUF tail with the next step's weight loads.
SLOTS = [(0, 3), (0, 2), (0, 0), (1, 0), (1, 3), (2, 0), (1, 2), (3, 0),
         (3, 1), (0, 1), (2, 1), (1, 1), (3, 2), (2, 2), (2, 3), (3, 3)]
ACT_AFTER = {7: 0, 11: 1, 13: 2, 15: 3}   # after slot q, activate chunk j

# module-level stash for test harness introspection
last_run_info = {}


def _dap(handle, offset, dims):
    """Explicit AP on a DRAM tensor: dims = [[step, count], ...] (element
    units; first dim pairs with the SBUF partition dim)."""
    base = handle[:]
    return bass.AP(tensor=base.tensor, offset=offset, ap=[list(d) for d in dims])


def build_nc(T=TFULL, tblk=TBLK):
    """Emit the per-core Bass/Tile program (SPMD; all cores identical)."""
    assert T % 512 == 0, T
    nblk = T // tblk
    BT = BC * T                       # rows of x per core
    th_n = T // 512                   # 512-wide tau halves in conv
    KCN = T // 128                    # lag chunks
    UPADW = 512 + T                   # zeros(512) ++ u(T)
    USHW = T + 384                    # Qi domain width

    MULT = mybir.AluOpType.mult
    MAX = mybir.AluOpType.max

    nc = bacc.Bacc(None, target_bir_lowering=False)
    xT_d = nc.declare_dram_parameter("xT", [FEAT, BT], BF16, isOutput=False)
    whT_d = nc.declare_dram_parameter("whT", [HID, HID], BF16, isOutput=False)
    g_d = nc.declare_dram_parameter("g", [T, HID], BF16, isOutput=False)
    wx_d = nc.declare_dram_parameter("wx", [FEAT, HID], BF16, isOutput=False)
    ex_d = nc.declare_dram_parameter("ex", [FEAT, 1], BF16, isOutput=False)
    out_d = nc.declare_dram_parameter("out", [BT, HID], BF16, isOutput=True)

    upad_d = nc.dram_tensor("u_pad", [BC, UPADW], BF16)

    with tile.TileContext(nc) as tc:
        with (
            tc.tile_pool(name="consts", bufs=1) as consts,
            tc.tile_pool(name="work", bufs=4) as work,
            tc.tile_pool(name="hout", bufs=2) as hpool,
            tc.tile_pool(name="psA", bufs=4, space="PSUM") as psA,
            tc.tile_pool(name="psS", bufs=4, space="PSUM") as psS,
        ):
            # ---- resident constants -------------------------------------
            whT_sb = consts.tile([128, 4, HID], BF16)
            for kc in range(4):
                nc.sync.dma_start(out=whT_sb[:, kc, :],
                                  in_=whT_d[kc * 128:(kc + 1) * 128, :])
            g_sb = consts.tile([128, KCN, HID], BF16)
            for kc in range(KCN):
                nc.sync.dma_start(out=g_sb[:, kc, :],
                                  in_=g_d[kc * 128:(kc + 1) * 128, :])
            wx_sb = consts.tile([128, HID], BF16)
            nc.sync.dma_start(out=wx_sb, in_=wx_d[:, :])
            ex_sb = consts.tile([128, 1], BF16)
            nc.sync.dma_start(out=ex_sb, in_=ex_d[:, :])
            xT_sb = consts.tile([128, BT], BF16)    # x.T : [feat, (b,tau)]
            nc.sync.dma_start(out=xT_sb, in_=xT_d[:, :])

            ushr = consts.tile([128, BC, USHW], BF16)  # reversed u shifts
            c_sb = consts.tile([128, 4, BC, T], BF16)  # c.T resident
            h0 = consts.tile([128, 4, BC], BF16)
            nc.vector.memset(h0, 0.0)

            # ---- phase B: u = x @ e_x  ->  u_pad DRAM -------------------
            for b8 in range(BC):
                urow = work.tile([1, UPADW], BF16, tag="urow")
                nc.vector.memset(urow[:, 0:512], 0.0)
                for th in range(th_n):
                    ps = psA.tile([1, 512], F32, tag="ps")
                    nc.tensor.matmul(ps, lhsT=ex_sb,
                                     rhs=xT_sb[:, b8 * T + th * 512:
                                               b8 * T + (th + 1) * 512],
                                     start=True, stop=True)
                    nc.scalar.copy(urow[:, 512 + th * 512:512 + (th + 1) * 512],
                                   ps)
                nc.gpsimd.dma_start(out=upad_d[b8:b8 + 1, :], in_=urow)

            # ---- phase C: reversed shift matrix -------------------------
            # ushr[p, b, Qi] = u_pad[b][1 + Qi + p]
            for b8 in range(BC):
                nc.gpsimd.dma_start(
                    out=ushr[:, b8, :],
                    in_=_dap(upad_d, b8 * UPADW + 1, [[1, 128], [1, USHW]]))

            # ---- phase D: c.T = conv(G, u) + W_x.T @ x.T -> c_sb --------
            ev = 0
            for jt in range(4):
                for th in range(th_n):
                    kmax = min(KCN, 4 * (th + 1))
                    for b8 in range(BC):
                        ps = psA.tile([128, 512], F32, tag="ps")
                        for kc in range(kmax):
                            qi0 = 384 + 512 * th - 128 * kc
                            nc.tensor.matmul(
                                ps, lhsT=g_sb[:, kc, jt * 128:(jt + 1) * 128],
                                rhs=ushr[:, b8, qi0:qi0 + 512],
                                start=(kc == 0), stop=False)
                        nc.tensor.matmul(
                            ps, lhsT=wx_sb[:, jt * 128:(jt + 1) * 128],
                            rhs=xT_sb[:, b8 * T + th * 512:b8 * T + (th + 1) * 512],
                            start=False, stop=True)
                        dst = c_sb[:, jt, b8, th * 512:(th + 1) * 512]
                        if ev % 2 == 0:
                            nc.scalar.copy(dst, ps)
                        else:
                            nc.vector.tensor_copy(dst, ps)
                        ev += 1

            # ---- phase E: sequential h recurrence -----------------------
            # Warm all psS banks once: a start=True pass sets has_written
            # over our [128, 4*BC] region so per-step matmuls can run
            # start=False and accumulate onto a prewritten c_t.
            warm = [psS.tile([128, 4, BC], F32, tag="pss", name=f"warm{i}")
                    for i in range(4)]
            for mc in range(4):
                for wt in warm:
                    nc.tensor.matmul(
                        wt[:, mc, :],
                        lhsT=whT_sb[:, 0, mc * 128:(mc + 1) * 128],
                        rhs=h0[:, 0, :],
                        start=(mc == 0), stop=(mc == 3),
                        skip_group_check=True)

            h_prev = h0                      # [128, 4(kc), BC] bf16
            h_prev_dt = None
            # prefetch c_0 into the first psum tile (ScalarE)
            ps_cur = psS.tile([128, 4, BC], F32, tag="pss")
            nc.scalar.copy(ps_cur, c_sb[:, :, :, 0])
            for blk in range(nblk):
                hb = hpool.tile([128, tblk, 4, BC], BF16, tag="hb")
                for dt in range(tblk):
                    t = blk * tblk + dt
                    ps = ps_cur
                    if t + 1 < T:
                        ps_cur = psS.tile([128, 4, BC], F32, tag="pss")
                        nc.scalar.copy(ps_cur, c_sb[:, :, :, t + 1])
                    else:
                        ps_cur = None
                    for q, (r, w) in enumerate(SLOTS):
                        rhs = (h_prev[:, r, :] if h_prev_dt is None
                               else h_prev[:, h_prev_dt, r, :])
                        nc.tensor.matmul(
                            ps[:, w, :],
                            lhsT=whT_sb[:, r, w * 128:(w + 1) * 128],
                            rhs=rhs,
                            start=False, stop=False,
                            skip_group_check=True)
                        j = ACT_AFTER.get(q)
                        if j is not None:
                            # leaky_relu(z) = max(z, 0.2 z), one DVE op
                            nc.vector.scalar_tensor_tensor(
                                out=hb[:, dt, j, :],
                                in0=ps[:, j, :], scalar=0.2, in1=ps[:, j, :],
                                op0=MULT, op1=MAX)
                    h_prev = hb
                    h_prev_dt = dt
                # write block to DRAM out: row r=(b*T+t0+dt), col=128*mc+p
                t0 = blk * tblk
                for b8 in range(BC):
                    nc.sync.dma_start(
                        out=_dap(out_d, (b8 * T + t0) * HID,
                                 [[1, 128], [HID, tblk], [128, 4]]),
                        in_=hb[:, :, :, b8])
    nc.compile()
    return nc


_nc_cache = {}


def _get_nc(T, tblk):
    key = (T, tblk)
    if key not in _nc_cache:
        _nc_cache[key] = build_nc(T, tblk)
    return _nc_cache[key]


def host_prep(x, A, Bv, W_x, e_x, W_h, W_m, T):
    """Host-side constant prep (float64, exact fn of constant inputs)."""
    order = A.shape[0]
    A64 = A.astype(np.float64)
    b64 = Bv[:, 0].astype(np.float64)
    Hk = np.empty((T, order))
    v = b64.copy()
    for k in range(T):
        Hk[k] = v
        v = A64 @ v
    G = (Hk @ W_m.T.astype(np.float64)).astype(np.float32)      # (T, 512)
    # reverse lag index within each 128-chunk (matches reversed u-shift rows)
    Gr = G.reshape(T // 128, 128, -1)[:, ::-1, :].reshape(T, -1)
    Gr = np.ascontiguousarray(Gr).astype(BF)
    whT = np.ascontiguousarray(W_h.T).astype(BF)
    return Gr, whT


def kernel(x, A, Bv, W_x, e_x, W_h, W_m, T=TFULL, tblk=TBLK):
    x = np.asarray(x, np.float32)
    A = np.asarray(A, np.float32)
    Bv = np.asarray(Bv, np.float32)
    W_x = np.asarray(W_x, np.float32)
    e_x = np.asarray(e_x, np.float32)
    W_h = np.asarray(W_h, np.float32)
    W_m = np.asarray(W_m, np.float32)

    Gr, whT = host_prep(x, A, Bv, W_x, e_x, W_h, W_m, T)
    wx16 = W_x.astype(BF)
    ex16 = e_x.astype(BF)

    nc = _get_nc(T, tblk)
    B = x.shape[0]
    in_maps = []
    for c in range(NCORES):
        xs = x[c * BC:(c + 1) * BC, 1:T + 1, :].reshape(BC * T, FEAT)
        xT = np.ascontiguousarray(xs.T).astype(BF)       # [feat, (b,tau)]
        in_maps.append({
            "xT": xT, "whT": whT, "g": Gr, "wx": wx16, "ex": ex16,
        })
    trace = bool(int(os.environ.get("KERNEL_TRACE", "0")))
    res = run_bass_kernel_spmd(nc, in_maps, list(range(NCORES)), trace=trace)
    last_run_info.clear()
    last_run_info.update(
        exec_time_ns=res.exec_time_ns,
        mean_exec_time_ns=res.mean_exec_time_ns,
        profile_json=res.profile_json,
    )
    out = np.empty((B, T, HID), np.float32)
    for c in range(NCORES):
        o = res.results[c]["out"].astype(np.float32).reshape(BC, T, HID)
        out[c * BC:(c + 1) * BC] = o
    return out


# revision 8
# speedup vs baseline: 9.3782x; 9.3782x over previous
# Trainium2 Bass kernel for the KerasLMU problem.
#
# Math: per time step t (T=1024 steps),
#   u_t = x_t @ e_x                       (B,1)
#   m_t = m_{t-1} @ A.T + b_row * u_t     (B,256)   -- linear recurrence
#   h_t = lrelu(x_t @ W_x + h_{t-1} @ W_h.T + m_t @ W_m.T)
#
# Reformulation:
#   m_t = sum_k A^k b u_{t-k}  (causal convolution), so
#   c_t := x_t @ W_x + m_t @ W_m.T = x_t @ W_x + sum_k G[k] u_{t-k}
#   with G[k] = W_m @ (A^k b) precomputed host-side in float64.
# The only sequential device work left is h_t = lrelu(c_t + h_{t-1} @ W_h.T).
#
# v2 changes vs baseline:
#   - whole conv pipeline in bf16 (fp32 matmul is 4 cyc/row on PE; bf16 is 1)
#   - c kept resident in SBUF (bf16, 64KB/partition): no cT DRAM roundtrip,
#     no per-block gather DMAs in the sequential phase
#   - x transposed host-side (no on-device transpose phase)
#   - sequential phase: 16 MMs/step issued in a latency-aware slot order
#     (each h-chunk finished early / read late), leaky-relu done per-chunk
#     as one DVE scalar_tensor_tensor (max(ps, 0.2*ps)), c prefetch for the
#     next step done on the otherwise-idle ScalarE.
#
# Sharding: data-parallel over batch. 64 batch rows -> 8 cores x 8 rows.
# All weights replicated; no collectives.

import os
import sys

sys.path.insert(0, "/opt/trn_rl_repo")

import numpy as np
import ml_dtypes

import concourse.bass as bass
import concourse.tile as tile
from concourse import bacc, mybir
from concourse.bass_utils import run_bass_kernel_spmd

F32 = mybir.dt.float32
BF16 = mybir.dt.bfloat16
BF = ml_dtypes.bfloat16

NCORES = 8
BATCH = 64
BC = BATCH // NCORES          # batch rows per core = 8
FEAT = 128
HID = 512
ORDER = 256
TFULL = 1024
TBLK = 64                     # seq-loop steps per output-DMA block

# Per-step matmul issue order: (read h-chunk r, write psum chunk w).
# Chunks 0+1 are fully written by slot 9 so the first half-activation
# (ScalarE Prelu on psum chunks 0:2) starts early and finishes before
# the second half's dependency (slot 15) -- no ScalarE serialization on
# the critical path.  Reads of chunks 2/3 sit at slots >= 6 so the
# second half-activation's tail overlaps the next step's weight loads.
SLOTS = [(0, 2), (0, 0), (1, 0), (0, 1), (1, 1), (1, 3), (2, 0), (3, 0),
         (2, 1), (3, 1), (0, 3), (1, 2), (2, 2), (3, 2), (2, 3), (3, 3)]
ACT_AFTER = {9: 0, 15: 2}   # after slot q, activate psum chunks [j, j+2)

# module-level stash for test harness introspection
last_run_info = {}


def _dap(handle, offset, dims):
    """Explicit AP on a DRAM tensor: dims = [[step, count], ...] (element
    units; first dim pairs with the SBUF partition dim)."""
    base = handle[:]
    return bass.AP(tensor=base.tensor, offset=offset, ap=[list(d) for d in dims])


def build_nc(T=TFULL, tblk=TBLK):
    """Emit the per-core Bass/Tile program (SPMD; all cores identical)."""
    assert T % 512 == 0, T
    nblk = T // tblk
    BT = BC * T                       # rows of x per core
    th_n = T // 512                   # 512-wide tau halves in conv
    KCN = T // 128                    # lag chunks
    UPADW = 512 + T                   # zeros(512) ++ u(T)
    USHW = T + 384                    # Qi domain width

    nc = bacc.Bacc(None, target_bir_lowering=False)
    xT_d = nc.declare_dram_parameter("xT", [FEAT, BT], BF16, isOutput=False)
    whT_d = nc.declare_dram_parameter("whT", [HID, HID], BF16, isOutput=False)
    g_d = nc.declare_dram_parameter("g", [T, HID], BF16, isOutput=False)
    wx_d = nc.declare_dram_parameter("wx", [FEAT, HID], BF16, isOutput=False)
    ex_d = nc.declare_dram_parameter("ex", [FEAT, 1], BF16, isOutput=False)
    # out in hb-native layout [p, (blk, dt, mc, b)]; host un-permutes.
    # A [b,t,j]-ordered layout would DMA 2 bytes per descriptor (4.2M
    # packets); this layout is 128 x 4KB contiguous runs per block.
    out_d = nc.declare_dram_parameter("out", [128, BT * HID // 128], BF16,
                                      isOutput=True)

    upad_d = nc.dram_tensor("u_pad", [BC, UPADW], BF16)

    with tile.TileContext(nc) as tc:
        with (
            tc.tile_pool(name="consts", bufs=1) as consts,
            tc.tile_pool(name="work", bufs=4) as work,
            tc.tile_pool(name="hout", bufs=2) as hpool,
            tc.tile_pool(name="psA", bufs=4, space="PSUM") as psA,
            tc.tile_pool(name="psS", bufs=4, space="PSUM") as psS,
        ):
            # ---- resident constants -------------------------------------
            whT_sb = consts.tile([128, 4, HID], BF16)
            for kc in range(4):
                nc.sync.dma_start(out=whT_sb[:, kc, :],
                                  in_=whT_d[kc * 128:(kc + 1) * 128, :])
            g_sb = consts.tile([128, KCN, HID], BF16)
            for kc in range(KCN):
                nc.sync.dma_start(out=g_sb[:, kc, :],
                                  in_=g_d[kc * 128:(kc + 1) * 128, :])
            wx_sb = consts.tile([128, HID], BF16)
            nc.sync.dma_start(out=wx_sb, in_=wx_d[:, :])
            ex_sb = consts.tile([128, 1], BF16)
            nc.sync.dma_start(out=ex_sb, in_=ex_d[:, :])
            xT_sb = consts.tile([128, BT], BF16)    # x.T : [feat, (b,tau)]
            nc.sync.dma_start(out=xT_sb, in_=xT_d[:, :])

            ushr = consts.tile([128, BC, USHW], BF16)  # reversed u shifts
            c_sb = consts.tile([128, 4, BC, T], BF16)  # c.T resident
            h0 = consts.tile([128, 4, BC], BF16)
            nc.vector.memset(h0, 0.0)

            # ---- phase B: u = x @ e_x  ->  u_pad DRAM -------------------
            for b8 in range(BC):
                urow = work.tile([1, UPADW], BF16, tag="urow")
                nc.vector.memset(urow[:, 0:512], 0.0)
                for th in range(th_n):
                    ps = psA.tile([1, 512], F32, tag="ps")
                    nc.tensor.matmul(ps, lhsT=ex_sb,
                                     rhs=xT_sb[:, b8 * T + th * 512:
                                               b8 * T + (th + 1) * 512],
                                     start=True, stop=True)
                    nc.scalar.copy(urow[:, 512 + th * 512:512 + (th + 1) * 512],
                                   ps)
                nc.gpsimd.dma_start(out=upad_d[b8:b8 + 1, :], in_=urow)

            # ---- phase C: reversed shift matrix -------------------------
            # ushr[p, b, Qi] = u_pad[b][1 + Qi + p]
            for b8 in range(BC):
                nc.gpsimd.dma_start(
                    out=ushr[:, b8, :],
                    in_=_dap(upad_d, b8 * UPADW + 1, [[1, 128], [1, USHW]]))

            # ---- phase D: c.T = conv(G, u) + W_x.T @ x.T -> c_sb --------
            ev = 0
            for jt in range(4):
                for th in range(th_n):
                    kmax = min(KCN, 4 * (th + 1))
                    for b8 in range(BC):
                        ps = psA.tile([128, 512], F32, tag="ps")
                        for kc in range(kmax):
                            qi0 = 384 + 512 * th - 128 * kc
                            nc.tensor.matmul(
                                ps, lhsT=g_sb[:, kc, jt * 128:(jt + 1) * 128],
                                rhs=ushr[:, b8, qi0:qi0 + 512],
                                start=(kc == 0), stop=False)
                        nc.tensor.matmul(
                            ps, lhsT=wx_sb[:, jt * 128:(jt + 1) * 128],
                            rhs=xT_sb[:, b8 * T + th * 512:b8 * T + (th + 1) * 512],
                            start=False, stop=True)
                        dst = c_sb[:, jt, b8, th * 512:(th + 1) * 512]
                        if ev % 2 == 0:
                            nc.scalar.copy(dst, ps)
                        else:
                            nc.vector.tensor_copy(dst, ps)
                        ev += 1

            # ---- phase E: sequential h recurrence -----------------------
            # Warm all psS banks once: a start=True pass sets has_written
            # over our [128, 4*BC] region so per-step matmuls can run
            # start=False and accumulate onto a prewritten c_t.
            warm = [psS.tile([128, 4, BC], F32, tag="pss", name=f"warm{i}")
                    for i in range(4)]
            for mc in range(4):
                for wt in warm:
                    nc.tensor.matmul(
                        wt[:, mc, :],
                        lhsT=whT_sb[:, 0, mc * 128:(mc + 1) * 128],
                        rhs=h0[:, 0, :],
                        start=(mc == 0), stop=(mc == 3),
                        skip_group_check=True)

            h_prev = h0                      # [128, 4(kc), BC] bf16
            h_prev_dt = None
            # prefetch c_0 into the first psum tile (DVE)
            ps_cur = psS.tile([128, 4, BC], F32, tag="pss")
            nc.vector.tensor_copy(ps_cur, c_sb[:, :, :, 0])
            for blk in range(nblk):
                hb = hpool.tile([128, tblk, 4, BC], BF16, tag="hb")
                for dt in range(tblk):
                    t = blk * tblk + dt
                    ps = ps_cur
                    if t + 1 < T:
                        ps_cur = psS.tile([128, 4, BC], F32, tag="pss")
                        nc.vector.tensor_copy(ps_cur, c_sb[:, :, :, t + 1])
                    else:
                        ps_cur = None
                    for q, (r, w) in enumerate(SLOTS):
                        rhs = (h_prev[:, r, :] if h_prev_dt is None
                               else h_prev[:, h_prev_dt, r, :])
                        nc.tensor.matmul(
                            ps[:, w, :],
                            lhsT=whT_sb[:, r, w * 128:(w + 1) * 128],
                            rhs=rhs,
                            start=False, stop=False,
                            skip_group_check=True)
                        j = ACT_AFTER.get(q)
                        if j is not None:
                            nc.scalar.activation(
                                hb[:, dt, j:j + 2, :],
                                ps[:, j:j + 2, :],
                                mybir.ActivationFunctionType.Prelu,
                                alpha=0.2)
                    h_prev = hb
                    h_prev_dt = dt
                # contiguous dump of the whole block tile; host un-permutes
                bw = tblk * 4 * BC
                nc.sync.dma_start(
                    out=out_d[:, blk * bw:(blk + 1) * bw],
                    in_=hb[:, :, :, :])
    nc.compile()
    return nc


_nc_cache = {}


def _get_nc(T, tblk):
    key = (T, tblk)
    if key not in _nc_cache:
        _nc_cache[key] = build_nc(T, tblk)
    return _nc_cache[key]


def host_prep(x, A, Bv, W_x, e_x, W_h, W_m, T):
    """Host-side constant prep (float64, exact fn of constant inputs)."""
    order = A.shape[0]
    A64 = A.astype(np.float64)
    b64 = Bv[:, 0].astype(np.float64)
    Hk = np.empty((T, order))
    v = b64.copy()
    for k in range(T):
        Hk[k] = v
        v = A64 @ v
    G = (Hk @ W_m.T.astype(np.float64)).astype(np.float32)      # (T, 512)
    # reverse lag index within each 128-chunk (matches reversed u-shift rows)
    Gr = G.reshape(T // 128, 128, -1)[:, ::-1, :].reshape(T, -1)
    Gr = np.ascontiguousarray(Gr).astype(BF)
    whT = np.ascontiguousarray(W_h.T).astype(BF)
    return Gr, whT


def kernel(x, A, Bv, W_x, e_x, W_h, W_m, T=TFULL, tblk=TBLK):
    x = np.asarray(x, np.float32)
    A = np.asarray(A, np.float32)
    Bv = np.asarray(Bv, np.float32)
    W_x = np.asarray(W_x, np.float32)
    e_x = np.asarray(e_x, np.float32)
    W_h = np.asarray(W_h, np.float32)
    W_m = np.asarray(W_m, np.float32)

    Gr, whT = host_prep(x, A, Bv, W_x, e_x, W_h, W_m, T)
    wx16 = W_x.astype(BF)
    ex16 = e_x.astype(BF)

    nc = _get_nc(T, tblk)
    B = x.shape[0]
    in_maps = []
    for c in range(NCORES):
        xs = x[c * BC:(c + 1) * BC, 1:T + 1, :].reshape(BC * T, FEAT)
        xT = np.ascontiguousarray(xs.T).astype(BF)       # [feat, (b,tau)]
        in_maps.append({
            "xT": xT, "whT": whT, "g": Gr, "wx": wx16, "ex": ex16,
        })
    trace = bool(int(os.environ.get("KERNEL_TRACE", "0")))
    res = run_bass_kernel_spmd(nc, in_maps, list(range(NCORES)), trace=trace)
    last_run_info.clear()
    last_run_info.update(
        exec_time_ns=res.exec_time_ns,
        mean_exec_time_ns=res.mean_exec_time_ns,
        profile_json=res.profile_json,
    )
    out = np.empty((B, T, HID), np.float32)
    nblk = T // tblk
    for c in range(NCORES):
        o = res.results[c]["out"].reshape(128, nblk, tblk, 4, BC)
        # [p, blk, dt, mc, b] -> [b, (blk, dt), (mc, p)]
        o = o.transpose(4, 1, 2, 3, 0).reshape(BC, T, HID)
        out[c * BC:(c + 1) * BC] = o.astype(np.float32)
    return out


# revision 15
# speedup vs baseline: 12.5843x; 1.3419x over previous
# Trainium2 Bass kernel for the KerasLMU problem.
#
# Math: per time step t (T=1024 steps),
#   u_t = x_t @ e_x                       (B,1)
#   m_t = m_{t-1} @ A.T + b_row * u_t     (B,256)   -- linear recurrence
#   h_t = lrelu(x_t @ W_x + h_{t-1} @ W_h.T + m_t @ W_m.T)
#
# Reformulation:
#   m_t = sum_k A^k b u_{t-k}  (causal convolution), so
#   c_t := x_t @ W_x + m_t @ W_m.T = x_t @ W_x + sum_k G[k] u_{t-k}
#   with G[k] = W_m @ (A^k b) precomputed host-side in float64.
# The only sequential device work left is h_t = lrelu(c_t + h_{t-1} @ W_h.T).
#
# v2 changes vs baseline:
#   - whole conv pipeline in bf16 (fp32 matmul is 4 cyc/row on PE; bf16 is 1)
#   - c kept resident in SBUF (bf16, 64KB/partition): no cT DRAM roundtrip,
#     no per-block gather DMAs in the sequential phase
#   - x transposed host-side (no on-device transpose phase)
#   - sequential phase: 16 MMs/step issued in a latency-aware slot order
#     (each h-chunk finished early / read late), leaky-relu done per-chunk
#     as one DVE scalar_tensor_tensor (max(ps, 0.2*ps)), c prefetch for the
#     next step done on the otherwise-idle ScalarE.
#
# Sharding: data-parallel over batch. 64 batch rows -> 8 cores x 8 rows.
# All weights replicated; no collectives.

import os
import sys

sys.path.insert(0, "/opt/trn_rl_repo")

import numpy as np
import ml_dtypes

import concourse.bass as bass
import concourse.tile as tile
from concourse import bacc, mybir
from concourse.bass_utils import run_bass_kernel_spmd

F32 = mybir.dt.float32
BF16 = mybir.dt.bfloat16
BF = ml_dtypes.bfloat16

NCORES = 8
BATCH = 64
BC = BATCH // NCORES          # batch rows per core = 8
FEAT = 128
HID = 512
ORDER = 256
TFULL = 1024
TBLK = 64                     # seq-loop steps per output-DMA block

# Per-step matmul issue order: (read h-chunk r, write psum chunk w).
# The 16 MMs issue at the NX dispatch floor (~26ns each); the step period
# is bound by the single full-width activation + semaphore hops, so one
# ACT (not two serialized halves) minimizes the ring.
SLOTS = [(0, 2), (0, 0), (1, 0), (0, 1), (1, 1), (1, 3), (2, 0), (3, 0),
         (2, 1), (3, 1), (0, 3), (1, 2), (2, 2), (3, 2), (2, 3), (3, 3)]

# module-level stash for test harness introspection
last_run_info = {}


def _dap(handle, offset, dims):
    """Explicit AP on a DRAM tensor: dims = [[step, count], ...] (element
    units; first dim pairs with the SBUF partition dim)."""
    base = handle[:]
    return bass.AP(tensor=base.tensor, offset=offset, ap=[list(d) for d in dims])


def build_nc(T=TFULL, tblk=TBLK, interleave=True):
    """Emit the per-core Bass/Tile program (SPMD; all cores identical)."""
    assert T % 512 == 0, T
    nblk = T // tblk
    BT = BC * T                       # rows of x per core
    th_n = T // 512                   # 512-wide tau halves in conv
    KCN = T // 128                    # lag chunks
    UPADW = 512 + T                   # zeros(512) ++ u(T)
    USHW = T + 384                    # Qi domain width

    nc = bacc.Bacc(None, target_bir_lowering=False)
    xT_d = nc.declare_dram_parameter("xT", [FEAT, BT], BF16, isOutput=False)
    whT_d = nc.declare_dram_parameter("whT", [HID, HID], BF16, isOutput=False)
    g_d = nc.declare_dram_parameter("g", [T, HID], BF16, isOutput=False)
    wx_d = nc.declare_dram_parameter("wx", [FEAT, HID], BF16, isOutput=False)
    ex_d = nc.declare_dram_parameter("ex", [FEAT, 1], BF16, isOutput=False)
    # out in hb-native layout [p, (blk, dt, mc, b)]; host un-permutes.
    # A [b,t,j]-ordered layout would DMA 2 bytes per descriptor (4.2M
    # packets); this layout is 128 x 4KB contiguous runs per block.
    out_d = nc.declare_dram_parameter("out", [128, BT * HID // 128], BF16,
                                      isOutput=True)

    upad_d = nc.dram_tensor("u_pad", [BC, UPADW], BF16)

    with tile.TileContext(nc) as tc:
        with (
            tc.tile_pool(name="consts", bufs=1) as consts,
            tc.tile_pool(name="work", bufs=4) as work,
            tc.tile_pool(name="hout", bufs=2) as hpool,
            tc.tile_pool(name="psA", bufs=4, space="PSUM") as psA,
            tc.tile_pool(name="psS", bufs=4, space="PSUM") as psS,
        ):
            # ---- resident constants -------------------------------------
            whT_sb = consts.tile([128, 4, HID], BF16)
            for kc in range(4):
                nc.sync.dma_start(out=whT_sb[:, kc, :],
                                  in_=whT_d[kc * 128:(kc + 1) * 128, :])
            g_sb = consts.tile([128, KCN, HID], BF16)
            for kc in range(KCN):
                nc.sync.dma_start(out=g_sb[:, kc, :],
                                  in_=g_d[kc * 128:(kc + 1) * 128, :])
            wx_sb = consts.tile([128, HID], BF16)
            nc.sync.dma_start(out=wx_sb, in_=wx_d[:, :])
            ex_sb = consts.tile([128, 1], BF16)
            nc.sync.dma_start(out=ex_sb, in_=ex_d[:, :])
            xT_sb = consts.tile([128, BT], BF16)    # x.T : [feat, (b,tau)]
            nc.sync.dma_start(out=xT_sb, in_=xT_d[:, :])

            ushr = consts.tile([128, BC, USHW], BF16)  # reversed u shifts
            c_sb = consts.tile([128, 4, BC, T], BF16)  # c.T resident
            h0 = consts.tile([128, 4, BC], BF16)
            nc.vector.memset(h0, 0.0)

            # ---- phase B: u = x @ e_x  ->  u_pad DRAM -------------------
            for b8 in range(BC):
                urow = work.tile([1, UPADW], BF16, tag="urow")
                nc.vector.memset(urow[:, 0:512], 0.0)
                for th in range(th_n):
                    ps = psA.tile([1, 512], F32, tag="ps")
                    nc.tensor.matmul(ps, lhsT=ex_sb,
                                     rhs=xT_sb[:, b8 * T + th * 512:
                                               b8 * T + (th + 1) * 512],
                                     start=True, stop=True)
                    nc.scalar.copy(urow[:, 512 + th * 512:512 + (th + 1) * 512],
                                   ps)
                nc.gpsimd.dma_start(out=upad_d[b8:b8 + 1, :], in_=urow)

            # ---- phase C: reversed shift matrix -------------------------
            # ushr[p, b, Qi] = u_pad[b][1 + Qi + p]
            for b8 in range(BC):
                nc.gpsimd.dma_start(
                    out=ushr[:, b8, :],
                    in_=_dap(upad_d, b8 * UPADW + 1, [[1, 128], [1, USHW]]))

            # ---- phase D prelude: tau-half 0 of c (needed from step 0) --
            # Runs the PE back-to-back (warms HAM before the ridge loop).
            ev = 0
            for jt in range(4):
                for b8 in range(BC):
                    ps = psA.tile([128, 512], F32, tag="ps")
                    for kc in range(4):
                        qi0 = 384 - 128 * kc
                        nc.tensor.matmul(
                            ps, lhsT=g_sb[:, kc, jt * 128:(jt + 1) * 128],
                            rhs=ushr[:, b8, qi0:qi0 + 512],
                            start=(kc == 0), stop=False)
                    nc.tensor.matmul(
                        ps, lhsT=wx_sb[:, jt * 128:(jt + 1) * 128],
                        rhs=xT_sb[:, b8 * T:b8 * T + 512],
                        start=False, stop=True)
                    dst = c_sb[:, jt, b8, 0:512]
                    if ev % 2 == 0:
                        nc.scalar.copy(dst, ps)
                    else:
                        nc.vector.tensor_copy(dst, ps)
                    ev += 1

            # ---- phase D remainder: tau-halves >= 1, interleaved --------
            # into phase E's activation-wait windows as N=256 half-MMs
            # (2 per step).  Each emitted item is a closure; groups of 9
            # logical MMs accumulate into one psA tile, then a DVE copy
            # moves it into c_sb.  Group g finishes by step ~9g+8; all
            # done by step ~290, well before step 512 consumes them.
            d_items = []
            for th in range(1, th_n):
                kmax = min(KCN, 4 * (th + 1))
                for jt in range(4):
                    for b8 in range(BC):
                        d_items.append(("group", th, kmax, jt, b8))
            d_queue = []
            if not interleave:
                for _, th, kmax, jt, b8 in d_items:
                    ps = psA.tile([128, 512], F32, tag="ps")
                    for kc in range(kmax):
                        qi0 = 384 + 512 * th - 128 * kc
                        nc.tensor.matmul(
                            ps, lhsT=g_sb[:, kc, jt * 128:(jt + 1) * 128],
                            rhs=ushr[:, b8, qi0:qi0 + 512],
                            start=(kc == 0), stop=False)
                    nc.tensor.matmul(
                        ps, lhsT=wx_sb[:, jt * 128:(jt + 1) * 128],
                        rhs=xT_sb[:, b8 * T + th * 512:b8 * T + (th + 1) * 512],
                        start=False, stop=True)
                    dst = c_sb[:, jt, b8, th * 512:(th + 1) * 512]
                    if ev % 2 == 0:
                        nc.scalar.copy(dst, ps)
                    else:
                        nc.vector.tensor_copy(dst, ps)
                    ev += 1
                d_items = []

            def emit_d_mm():
                if not d_queue and d_items:
                    g = d_items.pop(0)
                    _, th, kmax, jt, b8 = g
                    ps_d = psA.tile([128, 512], F32, tag="ps")
                    # kc=0 full-width with start=True (initializes the whole
                    # tile's has_written in one shot -- a start=True on a
                    # half would wipe the other half's accumulation bits)
                    d_queue.append(("mm", ps_d, g, 0, True, None, 384 + 512 * th))
                    for kc in range(1, kmax):
                        qi0 = 384 + 512 * th - 128 * kc
                        for half in range(2):
                            d_queue.append(
                                ("mm", ps_d, g, kc, False, half, qi0))
                    for half in range(2):
                        d_queue.append(("mm", ps_d, g, -1, False, half, 0))
                    d_queue.append(("copy", ps_d, g))
                if not d_queue:
                    return
                item = d_queue.pop(0)
                if item[0] == "copy":
                    _, ps_d, (_, th, _, jt, b8) = item
                    nc.vector.tensor_copy(
                        c_sb[:, jt, b8, th * 512:(th + 1) * 512], ps_d)
                    return
                _, ps_d, (_, th, kmax, jt, b8), kc, first, half, qi0 = item
                o = 0 if half is None else half * 256
                n = 512 if half is None else 256
                if kc >= 0:
                    nc.tensor.matmul(
                        ps_d[:, o:o + n],
                        lhsT=g_sb[:, kc, jt * 128:(jt + 1) * 128],
                        rhs=ushr[:, b8, qi0 + o:qi0 + o + n],
                        start=first, stop=False, skip_group_check=True)
                else:
                    nc.tensor.matmul(
                        ps_d[:, o:o + n],
                        lhsT=wx_sb[:, jt * 128:(jt + 1) * 128],
                        rhs=xT_sb[:, b8 * T + th * 512 + o:
                                  b8 * T + th * 512 + o + n],
                        start=False, stop=True, skip_group_check=True)

            # ---- phase E: sequential h recurrence -----------------------
            # Warm all psS banks once: a start=True pass sets has_written
            # over our [128, 4*BC] region so per-step matmuls can run
            # start=False and accumulate onto a prewritten c_t.
            warm = [psS.tile([128, 4, BC], F32, tag="pss", name=f"warm{i}")
                    for i in range(4)]
            for mc in range(4):
                for wt in warm:
                    nc.tensor.matmul(
                        wt[:, mc, :],
                        lhsT=whT_sb[:, 0, mc * 128:(mc + 1) * 128],
                        rhs=h0[:, 0, :],
                        start=(mc == 0), stop=(mc == 3),
                        skip_group_check=True)

            h_prev = h0                      # [128, 4(kc), BC] bf16
            h_prev_dt = None
            # prefetch c_0 into the first psum tile (DVE)
            ps_cur = psS.tile([128, 4, BC], F32, tag="pss")
            nc.vector.tensor_copy(ps_cur, c_sb[:, :, :, 0])
            for blk in range(nblk):
                hb = hpool.tile([128, tblk, 4, BC], BF16, tag="hb")
                for dt in range(tblk):
                    t = blk * tblk + dt
                    ps = ps_cur
                    if t + 1 < T:
                        ps_cur = psS.tile([128, 4, BC], F32, tag="pss")
                        nc.vector.tensor_copy(ps_cur, c_sb[:, :, :, t + 1])
                    else:
                        ps_cur = None
                    for q, (r, w) in enumerate(SLOTS):
                        rhs = (h_prev[:, r, :] if h_prev_dt is None
                               else h_prev[:, h_prev_dt, r, :])
                        nc.tensor.matmul(
                            ps[:, w, :],
                            lhsT=whT_sb[:, r, w * 128:(w + 1) * 128],
                            rhs=rhs,
                            start=False, stop=False,
                            skip_group_check=True)
                    nc.scalar.activation(
                        hb[:, dt, :, :], ps,
                        mybir.ActivationFunctionType.Prelu, alpha=0.2)
                    # fill the activation-wait window with leftover conv MMs
                    emit_d_mm()
                    emit_d_mm()
                    h_prev = hb
                    h_prev_dt = dt
                # contiguous dump of the whole block tile; host un-permutes
                bw = tblk * 4 * BC
                nc.sync.dma_start(
                    out=out_d[:, blk * bw:(blk + 1) * bw],
                    in_=hb[:, :, :, :])
    nc.compile()
    return nc


_nc_cache = {}


def _get_nc(T, tblk):
    interleave = bool(int(os.environ.get("KERNEL_INTERLEAVE", "1")))
    key = (T, tblk, interleave)
    if key not in _nc_cache:
        _nc_cache[key] = build_nc(T, tblk, interleave)
    return _nc_cache[key]


def host_prep(x, A, Bv, W_x, e_x, W_h, W_m, T):
    """Host-side constant prep (float64, exact fn of constant inputs)."""
    order = A.shape[0]
    A64 = A.astype(np.float64)
    b64 = Bv[:, 0].astype(np.float64)
    Hk = np.empty((T, order))
    v = b64.copy()
    for k in range(T):
        Hk[k] = v
        v = A64 @ v
    G = (Hk @ W_m.T.astype(np.float64)).astype(np.float32)      # (T, 512)
    # reverse lag index within each 128-chunk (matches reversed u-shift rows)
    Gr = G.reshape(T // 128, 128, -1)[:, ::-1, :].reshape(T, -1)
    Gr = np.ascontiguousarray(Gr).astype(BF)
    whT = np.ascontiguousarray(W_h.T).astype(BF)
    return Gr, whT


def kernel(x, A, Bv, W_x, e_x, W_h, W_m, T=TFULL, tblk=TBLK):
    x = np.asarray(x, np.float32)
    A = np.asarray(A, np.float32)
    Bv = np.asarray(Bv, np.float32)
    W_x = np.asarray(W_x, np.float32)
    e_x = np.asarray(e_x, np.float32)
    W_h = np.asarray(W_h, np.float32)
    W_m = np.asarray(W_m, np.float32)

    Gr, whT = host_prep(x, A, Bv, W_x, e_x, W_h, W_m, T)
    wx16 = W_x.astype(BF)
    ex16 = e_x.astype(BF)

    nc = _get_nc(T, tblk)
    B = x.shape[0]
    in_maps = []
    for c in range(NCORES):
        xs = x[c * BC:(c + 1) * BC, 1:T + 1, :].reshape(BC * T, FEAT)
        xT = np.ascontiguousarray(xs.T).astype(BF)       # [feat, (b,tau)]
        in_maps.append({
            "xT": xT, "whT": whT, "g": Gr, "wx": wx16, "ex": ex16,
        })
    trace = bool(int(os.environ.get("KERNEL_TRACE", "0")))
    res = run_bass_kernel_spmd(nc, in_maps, list(range(NCORES)), trace=trace)
    last_run_info.clear()
    last_run_info.update(
        exec_time_ns=res.exec_time_ns,
        mean_exec_time_ns=res.mean_exec_time_ns,
        profile_json=res.profile_json,
    )
    out = np.empty((B, T, HID), np.float32)
    nblk = T // tblk
    for c in range(NCORES):
        o = res.results[c]["out"].reshape(128, nblk, tblk, 4, BC)
        # [p, blk, dt, mc, b] -> [b, (blk, dt), (mc, p)]
        o = o.transpose(4, 1, 2, 3, 0).reshape(BC, T, HID)
        out[c * BC:(c + 1) * BC] = o.astype(np.float32)
    return out
